# revision 1
# baseline (speedup 1.0000x reference)
"""Trainium2 Bass kernel for a dense transformer block (B=2, T=2048, C=1024,
H=16 heads, HID=4096), sharded across 8 NeuronCores with no collectives.

Three SPMD launches (identical program on every core, per-core input data);
the host does the cross-core shuffles between launches (pure data movement:
slice/concat/transpose/relayout) plus the exact bias-vector adds bo/b2:
  L1 "qkv":  rows sharded (512 rows/core). LN1, PE-transpose of h, then
             Q^T/K^T in [head_dim, rows] layout and V in [rows, head_dim].
             Weights bf16, activations f32->bf16 hT, fp32 outputs.
  L2 "attn": heads sharded (2 heads/core, all B*T rows -> identical causal
             structure per core). Flash-style: S^T tiles [128 kv, 1024 q]
             via f32r matmuls, single Exp per tile over the live span,
             gpsimd affine_select for the diagonal mask, V|ones augmented
             matmul accumulates [att^T | denom] in PSUM, PE K=1 broadcast
             for 1/denom. Outputs att^T [128, B*T] in bf16.
  L3 "ffn":  rows sharded. att @ Wo (+x+bo residual), LN2, transpose,
             FFN1 relu(W1.T @ h2T + b1) kept in SBUF, FFN2 in two
             column-half passes of 4 accumulating PSUM banks, + residual.
             Weights/activations bf16, residual path fp32.

Numerics: matmuls f32r (~3.5e-4) or bf16; end-to-end rel err ~2e-3.
Cost-model time/core: L1 ~69us + L2 ~100us + L3 ~162us  (~8.6 GFLOP/core
of the ~17 useful GFLOP/core runs at the PE roofline in L3; L2 is
ACT(exp)-bound; L1 is DMA-bound).
"""

import sys

if "/opt/trn_rl_repo" not in sys.path:
    sys.path.insert(0, "/opt/trn_rl_repo")

import ml_dtypes
import numpy as np

import concourse.bass as bass
import concourse.mybir as mybir
import concourse.tile as tile
from concourse.bass_utils import run_bass_kernel_spmd
from concourse.masks import make_identity

F32 = mybir.dt.float32
F32R = mybir.dt.float32r
BF16 = mybir.dt.bfloat16
AF = mybir.ActivationFunctionType
ALU = mybir.AluOpType

B, T, C = 2, 2048, 1024
H, DH = 16, 64
HID = 4096
EPS = 1e-5
NC_ = 8
ROWS = (B * T) // NC_  # 512 rows per core
SCALE = C ** -0.5      # 1/32, matches reference

TRACE = False
LAST_EXEC_NS = []
LAST_RESULTS = []

_ev_ctr = [0]


def _legalize_waits(nc, max_waits=1):
    """This walrus build rejects instructions carrying >1 sync wait; split
    extra waits into standalone InstEventSemaphore carriers."""
    n = 0
    for f in nc.m.functions:
        for bb in f.blocks:
            insts = list(bb.instructions)
            if not any(
                i.sync_info is not None
                and i.sync_info.on_wait
                and len(i.sync_info.on_wait) > max_waits
                for i in insts
            ):
                continue
            new = []
            for ins in insts:
                si = ins.sync_info
                if si is not None and si.on_wait and len(si.on_wait) > max_waits:
                    waits = list(si.on_wait)
                    extra, keep = waits[:-max_waits], waits[-max_waits:]
                    for w in extra:
                        _ev_ctr[0] += 1
                        new.append(
                            mybir.InstEventSemaphore(
                                name=f"I-evw{_ev_ctr[0]}",
                                engine=ins.engine,
                                sync_info=mybir.SyncInfo(on_wait=[w], on_update=[]),
                            )
                        )
                        n += 1
                    ins.sync_info = mybir.SyncInfo(
                        on_wait=keep, on_update=list(si.on_update or [])
                    )
                new.append(ins)
            bb.instructions = new
    return n


def _layernorm_tile(nc, pools, xt, gbc, bbc, h_out):
    """LayerNorm along the free axis of xt [128, C]; h_out [128, C] f32.
    gbc/bbc are [128, C] broadcast gamma/beta tiles."""
    st = pools["stats"]
    s = st.tile([128, 1], F32, tag="s")
    nc.vector.tensor_reduce(s[:], xt[:], mybir.AxisListType.X, ALU.add)
    mu = st.tile([128, 1], F32, tag="mu")
    nc.scalar.mul(mu[:], s[:], 1.0 / C)
    sq = pools["scratch"].tile([128, C], F32)
    ss = st.tile([128, 1], F32, tag="ss")
    nc.scalar.activation(sq[:], xt[:], AF.Square, accum_out=ss[:])
    mu2 = st.tile([128, 1], F32, tag="mu2")
    nc.vector.tensor_mul(mu2[:], mu[:], mu[:])
    var = st.tile([128, 1], F32, tag="var")
    nc.vector.scalar_tensor_tensor(
        var[:], ss[:], 1.0 / C, mu2[:], op0=ALU.mult, op1=ALU.subtract
    )
    vare = st.tile([128, 1], F32, tag="vare")
    nc.vector.tensor_scalar_add(vare[:], var[:], EPS)
    std = st.tile([128, 1], F32, tag="std")
    nc.scalar.activation(std[:], vare[:], AF.Sqrt)
    rsig = st.tile([128, 1], F32, tag="rsig")
    nc.vector.reciprocal(rsig[:], std[:])
    nmr = st.tile([128, 1], F32, tag="nmr")
    nc.vector.scalar_tensor_tensor(
        nmr[:], mu[:], -1.0, rsig[:], op0=ALU.mult, op1=ALU.mult
    )
    h0 = pools["scratch"].tile([128, C], F32, tag="h0")
    nc.scalar.activation(h0[:], xt[:], AF.Identity, bias=nmr[:], scale=rsig[:])
    h1 = pools["scratch"].tile([128, C], F32, tag="h1")
    nc.vector.tensor_mul(h1[:], h0[:], gbc[:])
    nc.vector.tensor_add(h_out[:], h1[:], bbc[:])


def _build_l1():
    nc = bass.Bass()
    x = nc.declare_dram_parameter("x", [ROWS, C], F32, isOutput=False)
    gb = nc.declare_dram_parameter("ln_g", [128, C], F32, isOutput=False)
    bb_ = nc.declare_dram_parameter("ln_b", [128, C], F32, isOutput=False)
    wq = nc.declare_dram_parameter("wq", [C, C], BF16, isOutput=False)
    wk = nc.declare_dram_parameter("wk", [C, C], BF16, isOutput=False)
    wv = nc.declare_dram_parameter("wv", [C, C], BF16, isOutput=False)
    qt = nc.declare_dram_parameter("qt", [C, ROWS], F32, isOutput=True)
    kt = nc.declare_dram_parameter("kt", [C, ROWS], F32, isOutput=True)
    v = nc.declare_dram_parameter("v", [ROWS, C], F32, isOutput=True)

    with tile.TileContext(nc) as tc:
        import contextlib

        with contextlib.ExitStack() as ctx:
            const = ctx.enter_context(tc.tile_pool(name="const", bufs=1))
            xp = ctx.enter_context(tc.tile_pool(name="xp", bufs=4))
            hp = ctx.enter_context(tc.tile_pool(name="hp", bufs=4))
            htp = ctx.enter_context(tc.tile_pool(name="htp", bufs=1))
            wp = ctx.enter_context(tc.tile_pool(name="wp", bufs=10))
            op = ctx.enter_context(tc.tile_pool(name="op", bufs=4))
            scratch = ctx.enter_context(tc.tile_pool(name="scratch", bufs=2))
            stats = ctx.enter_context(tc.tile_pool(name="stats", bufs=2))
            tpsum = ctx.enter_context(tc.tile_pool(name="tpsum", bufs=2, space="PSUM"))
            mpsum = ctx.enter_context(tc.tile_pool(name="mpsum", bufs=3, space="PSUM"))
            pools = {"scratch": scratch, "stats": stats}

            # LN1 + transpose h -> hT [C, ROWS] stored [128, 8*512] k-tile major
            hT = htp.tile([128, 8 * ROWS], BF16)
            n_rt = ROWS // 128  # 4
            xts = []
            for rt in range(n_rt):
                xt = xp.tile([128, C], F32, tag="xt", name=f"xt_{rt}")
                nc.sync.dma_start(xt[:], x[rt * 128 : (rt + 1) * 128, :])
                xts.append(xt)
            ident = const.tile([128, 128], F32)
            make_identity(nc, ident[:])
            gbc = const.tile([128, C], F32, tag="gbc")
            nc.sync.dma_start(gbc[:], gb[:])
            bbc = const.tile([128, C], F32, tag="bbc")
            nc.sync.dma_start(bbc[:], bb_[:])
            hts = []
            for rt in range(n_rt):
                h = hp.tile([128, C], F32, tag="h", name=f"h_{rt}")
                _layernorm_tile(nc, pools, xts[rt], gbc, bbc, h)
                hts.append(h)
            # ct-outer so each hT column completes early and the k-accumulating
            # matmuls below can start while later columns still transpose
            for ct in range(8):
                for rt in range(n_rt):
                    tp = tpsum.tile([128, 128], F32)
                    nc.tensor.transpose(
                        tp[:], hts[rt][:, ct * 128 : (ct + 1) * 128], ident[:]
                    )
                    nc.vector.tensor_copy(
                        hT[:, ct * ROWS + rt * 128 : ct * ROWS + (rt + 1) * 128], tp[:]
                    )

            # Q^T / K^T = W.T @ hT   out [C(dims), ROWS]
            for w_ap, out_ap in ((wq, qt), (wk, kt)):
                for mtg in range(2):
                    wt = []
                    for ktile in range(8):
                        wtile = wp.tile([128, 512], BF16, tag="wt")
                        nc.sync.dma_start(
                            wtile[:],
                            w_ap[ktile * 128 : (ktile + 1) * 128,
                                 mtg * 512 : (mtg + 1) * 512],
                        )
                        wt.append(wtile)
                    for mti in range(4):
                        ps = mpsum.tile([128, ROWS], F32)
                        for ktile in range(8):
                            nc.tensor.matmul(
                                ps[:],
                                wt[ktile][:, mti * 128 : (mti + 1) * 128],
                                hT[:, ktile * ROWS : (ktile + 1) * ROWS],
                                start=(ktile == 0),
                                stop=(ktile == 7),
                            )
                        ot = op.tile([128, ROWS], F32)
                        nc.scalar.copy(ot[:], ps[:])
                        mt = mtg * 4 + mti
                        nc.sync.dma_start(out_ap[mt * 128 : (mt + 1) * 128, :], ot[:])

            # V = hT.T @ Wv   out [ROWS, C]
            for nh in range(2):
                wt = []
                for ktile in range(8):
                    wtile = wp.tile([128, 512], BF16, tag="wvt")
                    nc.sync.dma_start(
                        wtile[:],
                        wv[ktile * 128 : (ktile + 1) * 128, nh * 512 : (nh + 1) * 512],
                    )
                    wt.append(wtile)
                for rt in range(n_rt):
                    ps = mpsum.tile([128, 512], F32, tag="psv")
                    for ktile in range(8):
                        nc.tensor.matmul(
                            ps[:],
                            hT[:, ktile * ROWS + rt * 128 : ktile * ROWS + (rt + 1) * 128],
                            wt[ktile][:],
                            start=(ktile == 0),
                            stop=(ktile == 7),
                        )
                    ot = op.tile([128, 512], F32, tag="otv")
                    nc.scalar.copy(ot[:], ps[:])
                    nc.sync.dma_start(
                        v[rt * 128 : (rt + 1) * 128, nh * 512 : (nh + 1) * 512], ot[:]
                    )

    return nc


def _build_l2():
    """Per core: heads (2c, 2c+1), all B*T rows. Causal attention.
    qt2/kt2: [128, B*T] rows = 2 heads x 64 dims, cols = (b, t) flattened.
    vaug:    [128, 4*16*65]: per (b, hl) section of 16 kv-tiles [128, 64+1(ones)].
    out attt [128, B*T]."""
    BT = B * T
    nc = bass.Bass()
    qt2 = nc.declare_dram_parameter("qt2", [128, BT], F32R, isOutput=False)
    kt2 = nc.declare_dram_parameter("kt2", [128, BT], F32R, isOutput=False)
    vaug = nc.declare_dram_parameter("vaug", [128, 4 * 16 * 65], F32R, isOutput=False)
    ones64 = nc.declare_dram_parameter("ones64", [1, 64], F32R, isOutput=False)
    attt = nc.declare_dram_parameter("attt", [128, BT], BF16, isOutput=True)

    n_j = T // 512  # 4 q-blocks per batch

    with tile.TileContext(nc) as tc:
        import contextlib

        with contextlib.ExitStack() as ctx:
            const = ctx.enter_context(tc.tile_pool(name="const", bufs=1))
            big = ctx.enter_context(tc.tile_pool(name="big", bufs=1))
            ptp = ctx.enter_context(tc.tile_pool(name="ptp", bufs=4))
            small = ctx.enter_context(tc.tile_pool(name="small", bufs=3))
            spsum = ctx.enter_context(tc.tile_pool(name="spsum", bufs=2, space="PSUM"))
            apsum = ctx.enter_context(tc.tile_pool(name="apsum", bufs=2, space="PSUM"))
            rpsum = ctx.enter_context(tc.tile_pool(name="rpsum", bufs=2, space="PSUM"))

            qts = big.tile([128, BT], F32R, tag="qts")
            kts = big.tile([128, BT], F32R, tag="kts")
            vs = big.tile([128, 4 * 16 * 65], F32R, tag="vs")
            ones = const.tile([1, 64], F32R)
            nc.sync.dma_start(ones[:], ones64[:])
            # chunked loads in consumption order so the first (b, hl) block
            # starts as soon as its slices land
            for b in range(B):
                for Jc in range(2):
                    cs = slice(b * T + Jc * 1024, b * T + (Jc + 1) * 1024)
                    nc.sync.dma_start(kts[:, cs], kt2[:, cs])
                    nc.sync.dma_start(qts[:, cs], qt2[:, cs])
                vsec = slice((b * 2) * 16 * 65, (b * 2 + 2) * 16 * 65)
                nc.sync.dma_start(vs[:, vsec], vaug[:, vsec])
            att_sb = big.tile([128, BT], BF16, tag="att")
            zsrc = const.tile([128, 384], F32, tag="zsrc")
            nc.vector.memset(zsrc[:], 0.0)

            # q mega-blocks of 1024 (two 512 halves), kv tiles of 128.
            # Per (J, t): compute only the halves whose q range can attend
            # this kv tile; one Exp over the contiguous computed span.
            for b in range(B):
                for hl in range(2):
                    sec = b * 2 + hl
                    hlo = hl * 64
                    for J in range(T // 1024):  # 2
                        nkv = 8 * (J + 1)
                        aps = []
                        for h in range(2):
                            aps.append(
                                apsum.tile([128, 512], F32, tag="ap",
                                           name=f"ap_{sec}_{J}_{h}")
                            )
                        nlast = [8 * J + 4 * (h + 1) - 1 for h in range(2)]
                        for t in range(nkv):
                            # half h covers q [J*1024 + h*512, +512)
                            halves = [
                                h
                                for h in range(2)
                                if t * 128 < J * 1024 + (h + 1) * 512
                            ]
                            h0, h1 = halves[0], halves[-1]
                            span = slice(h0 * 512, (h1 + 1) * 512)
                            sp = spsum.tile([128, 1024], F32)
                            for h in halves:
                                nc.tensor.matmul(
                                    sp[:, h * 512 : (h + 1) * 512],
                                    kts[hlo : hlo + 64,
                                        b * T + t * 128 : b * T + (t + 1) * 128],
                                    qts[hlo : hlo + 64,
                                        b * T + J * 1024 + h * 512 :
                                        b * T + J * 1024 + (h + 1) * 512],
                                    start=True,
                                    stop=True,
                                )
                            pt = ptp.tile([128, 1024], F32R, tag="pt")
                            # diagonal masking: keep q_global >= kv_global.
                            # cols < off inside the diagonal half are fully
                            # masked: zero them and exp only the live span.
                            dh = t // 4 - 2 * J  # half index whose range contains kv
                            espan = span
                            if 0 <= dh < 2:
                                off = t * 128 - (J * 1024 + dh * 512)
                                if off > 0:
                                    nc.vector.tensor_copy(
                                        pt[:, dh * 512 : dh * 512 + off],
                                        zsrc[:, :off],
                                    )
                                espan = slice(dh * 512 + off, span.stop)
                            nc.scalar.activation(
                                pt[:, espan], sp[:, espan], AF.Exp, scale=SCALE
                            )
                            if 0 <= dh < 2:
                                nc.gpsimd.affine_select(
                                    pt[:, dh * 512 + off : (dh + 1) * 512],
                                    pt[:, dh * 512 + off : (dh + 1) * 512],
                                    pattern=[[1, 512 - off]],
                                    compare_op=ALU.is_ge,
                                    fill=0.0,
                                    base=0,
                                    channel_multiplier=-1,
                                )
                            for h in halves:
                                nc.tensor.matmul(
                                    aps[h][0:65, :],
                                    vs[:, (sec * 16 + t) * 65 :
                                       (sec * 16 + t) * 65 + 65],
                                    pt[:, h * 512 : (h + 1) * 512],
                                    start=(t == 0),
                                    stop=(t == nlast[h]),
                                )
                        for h in range(2):
                            # evict [att | den] once to SBUF (frees the PSUM
                            # accumulator), then normalize from SBUF
                            tmp = small.tile([65, 512], F32, tag="tmp")
                            nc.vector.tensor_copy(tmp[:], aps[h][0:65, :])
                            recr = small.tile([1, 512], F32R, tag="recr")
                            with nc.allow_low_precision(
                                reason="softmax denom reciprocal to f32r"
                            ):
                                nc.vector.reciprocal(recr[:], tmp[64:65, :])
                            rb = rpsum.tile([64, 512], F32)
                            nc.tensor.matmul(
                                rb[:], ones[0:1, :], recr[0:1, :],
                                start=True, stop=True,
                            )
                            nc.vector.tensor_mul(
                                att_sb[hlo : hlo + 64,
                                       b * T + J * 1024 + h * 512 :
                                       b * T + J * 1024 + (h + 1) * 512],
                                tmp[0:64, :],
                                rb[:],
                            )
            for b in range(B):
                for Jc in range(2):
                    cs = slice(b * T + Jc * 1024, b * T + (Jc + 1) * 1024)
                    nc.sync.dma_start(attt[:, cs], att_sb[:, cs])

    return nc


def _build_l3():
    nc = bass.Bass()
    attt = nc.declare_dram_parameter("attt", [128, 8 * ROWS], BF16, isOutput=False)
    xb = nc.declare_dram_parameter("xb", [ROWS, C], F32, isOutput=False)
    gb = nc.declare_dram_parameter("ln_g", [128, C], F32, isOutput=False)
    bb_ = nc.declare_dram_parameter("ln_b", [128, C], F32, isOutput=False)
    wo = nc.declare_dram_parameter("wo", [C, C], BF16, isOutput=False)
    w1 = nc.declare_dram_parameter("w1", [C, HID], BF16, isOutput=False)
    b1p = nc.declare_dram_parameter("b1", [128, HID // 128], F32, isOutput=False)
    w2 = nc.declare_dram_parameter("w2", [HID, C], BF16, isOutput=False)
    out = nc.declare_dram_parameter("out", [ROWS, C], F32, isOutput=True)

    n_rt = ROWS // 128  # 4
    n_hm = HID // 128  # 32

    with tile.TileContext(nc) as tc:
        import contextlib

        with contextlib.ExitStack() as ctx:
            const = ctx.enter_context(tc.tile_pool(name="const", bufs=1))
            x2p = ctx.enter_context(tc.tile_pool(name="x2p", bufs=4))
            h2tp = ctx.enter_context(tc.tile_pool(name="h2tp", bufs=1))
            atp = ctx.enter_context(tc.tile_pool(name="atp", bufs=n_hm))
            outp = ctx.enter_context(tc.tile_pool(name="outp", bufs=3))
            w1p = ctx.enter_context(tc.tile_pool(name="w1p", bufs=9))
            w2p = ctx.enter_context(tc.tile_pool(name="w2p", bufs=6))

            ident = const.tile([128, 128], F32)
            make_identity(nc, ident[:])
            gbc = const.tile([128, C], F32, tag="gbc")
            nc.sync.dma_start(gbc[:], gb[:])
            bbc = const.tile([128, C], F32, tag="bbc")
            nc.sync.dma_start(bbc[:], bb_[:])
            b1s = const.tile([128, HID // 128], F32, tag="b1s")
            nc.sync.dma_start(b1s[:], b1p[:])

            x2t = [x2p.tile([128, C], F32, tag="x2", name=f"x2_{i}") for i in range(n_rt)]
            h2T = h2tp.tile([128, 8 * ROWS], BF16)
            ats = []  # aT kept resident in SBUF, no DRAM roundtrip

            # phase A: proj + LN2 + transpose (pools closed before FFN2)
            with tc.tile_pool(name="attp", bufs=1) as attp, \
                 tc.tile_pool(name="wop", bufs=18) as wop, \
                 tc.tile_pool(name="xip", bufs=3) as xip, \
                 tc.tile_pool(name="scratch", bufs=2) as scratch, \
                 tc.tile_pool(name="stats", bufs=2) as stats, \
                 tc.tile_pool(name="hp", bufs=2) as hp, \
                 tc.tile_pool(name="mpsum", bufs=2, space="PSUM") as mpsum, \
                 tc.tile_pool(name="tpsum", bufs=2, space="PSUM") as tpsum:
                pools = {"scratch": scratch, "stats": stats}
                atts = attp.tile([128, 8 * ROWS], BF16)
                for ktile in range(8):
                    nc.sync.dma_start(
                        atts[:, ktile * ROWS : (ktile + 1) * ROWS],
                        attt[:, ktile * ROWS : (ktile + 1) * ROWS],
                    )
                wt2 = []
                for nh in range(2):
                    wt = []
                    for ktile in range(8):
                        wtile = wop.tile([128, 512], BF16, tag="wot",
                                         name=f"wot_{nh}_{ktile}")
                        nc.sync.dma_start(
                            wtile[:],
                            wo[ktile * 128 : (ktile + 1) * 128,
                               nh * 512 : (nh + 1) * 512],
                        )
                        wt.append(wtile)
                    wt2.append(wt)
                # x2 = att @ Wo + (x + bo); rt-outer so each row-tile's LN2
                # can start while later row-tiles still project
                for rt in range(n_rt):
                    for nh in range(2):
                        ps = mpsum.tile([128, 512], F32)
                        for ktile in range(8):
                            nc.tensor.matmul(
                                ps[:],
                                atts[:, ktile * ROWS + rt * 128 :
                                     ktile * ROWS + (rt + 1) * 128],
                                wt2[nh][ktile][:],
                                start=(ktile == 0),
                                stop=(ktile == 7),
                            )
                        xi = xip.tile([128, 512], F32)
                        nc.sync.dma_start(
                            xi[:],
                            xb[rt * 128 : (rt + 1) * 128, nh * 512 : (nh + 1) * 512],
                        )
                        nc.vector.tensor_add(
                            x2t[rt][:, nh * 512 : (nh + 1) * 512], ps[:], xi[:]
                        )
                    h = hp.tile([128, C], F32, tag="h", name=f"h2_{rt}")
                    _layernorm_tile(nc, pools, x2t[rt], gbc, bbc, h)
                    for ct in range(8):
                        tp = tpsum.tile([128, 128], F32)
                        nc.tensor.transpose(
                            tp[:], h[:, ct * 128 : (ct + 1) * 128], ident[:]
                        )
                        nc.vector.tensor_copy(
                            h2T[:, ct * ROWS + rt * 128 : ct * ROWS + (rt + 1) * 128],
                            tp[:],
                        )

            # FFN1 (aT = relu(W1.T @ h2T + b1) into SBUF) pipelined with
            # FFN2 (y = aT.T @ W2, two column-half passes of 4 PSUM banks)
            with tc.tile_pool(name="fpsum", bufs=2, space="PSUM") as fpsum, \
                 tc.tile_pool(name="ypsum", bufs=4, space="PSUM") as ypsum:
                for hm4 in range(HID // 512):  # 8
                    wt = []
                    for ktile in range(8):
                        wtile = w1p.tile([128, 512], BF16, tag="w1t")
                        nc.sync.dma_start(
                            wtile[:],
                            w1[ktile * 128 : (ktile + 1) * 128,
                               hm4 * 512 : (hm4 + 1) * 512],
                        )
                        wt.append(wtile)
                    for hmi in range(4):
                        hm = hm4 * 4 + hmi
                        ps = fpsum.tile([128, ROWS], F32)
                        for ktile in range(8):
                            nc.tensor.matmul(
                                ps[:],
                                wt[ktile][:, hmi * 128 : (hmi + 1) * 128],
                                h2T[:, ktile * ROWS : (ktile + 1) * ROWS],
                                start=(ktile == 0),
                                stop=(ktile == 7),
                            )
                        at = atp.tile([128, ROWS], BF16, tag="at", name=f"at_{hm}")
                        nc.scalar.activation(
                            at[:], ps[:], AF.Relu, bias=b1s[:, hm : hm + 1]
                        )
                        ats.append(at)

                for nh in range(2):
                    ys = [
                        ypsum.tile([128, 512], F32, tag="y", name=f"y_{nh}_{i}")
                        for i in range(n_rt)
                    ]
                    for hm in range(n_hm):
                        w2t = w2p.tile([128, 512], BF16, tag="w2t")
                        nc.sync.dma_start(
                            w2t[:],
                            w2[hm * 128 : (hm + 1) * 128, nh * 512 : (nh + 1) * 512],
                        )
                        for rt in range(n_rt):
                            nc.tensor.matmul(
                                ys[rt][:],
                                ats[hm][:, rt * 128 : (rt + 1) * 128],
                                w2t[:],
                                start=(hm == 0),
                                stop=(hm == n_hm - 1),
                            )
                    for rt in range(n_rt):
                        ot = outp.tile([128, 512], F32)
                        nc.vector.tensor_add(
                            ot[:], ys[rt][:], x2t[rt][:, nh * 512 : (nh + 1) * 512]
                        )
                        nc.sync.dma_start(
                            out[rt * 128 : (rt + 1) * 128, nh * 512 : (nh + 1) * 512],
                            ot[:],
                        )

    return nc


_PROGS = {}


def _progs():
    if not _PROGS:
        for name, build in (("l1", _build_l1), ("l2", _build_l2), ("l3", _build_l3)):
            nc = build()
            _legalize_waits(nc)
            _PROGS[name] = nc
    return _PROGS


def _run(nc, in_maps):
    kw = {}
    if TRACE:
        kw = dict(trace=True)
    res = run_bass_kernel_spmd(nc, in_maps, list(range(NC_)), **kw)
    if TRACE:
        LAST_EXEC_NS.append(res.exec_time_ns)
        LAST_RESULTS.append(res)
    return res.results


def kernel(x, ln1_g, ln1_b, Wq, Wk, Wv, Wo, bo, ln2_g, ln2_b, W1, b1, W2, b2):
    p = _progs()
    f32 = np.float32
    x = np.ascontiguousarray(np.asarray(x, f32))
    x_flat = x.reshape(B * T, C)

    bf16 = ml_dtypes.bfloat16
    wq_cat = np.ascontiguousarray(
        np.asarray(Wq, f32).transpose(1, 0, 2).reshape(C, C).astype(bf16))
    wk_cat = np.ascontiguousarray(
        np.asarray(Wk, f32).transpose(1, 0, 2).reshape(C, C).astype(bf16))
    wv_cat = np.ascontiguousarray(
        np.asarray(Wv, f32).transpose(1, 0, 2).reshape(C, C).astype(bf16))
    g1 = np.ascontiguousarray(np.broadcast_to(np.asarray(ln1_g, f32), (128, C)))
    b1v = np.ascontiguousarray(np.broadcast_to(np.asarray(ln1_b, f32), (128, C)))

    in1 = [
        {
            "x": np.ascontiguousarray(x_flat[c * ROWS : (c + 1) * ROWS]),
            "ln_g": g1,
            "ln_b": b1v,
            "wq": wq_cat,
            "wk": wk_cat,
            "wv": wv_cat,
        }
        for c in range(NC_)
    ]
    r1 = _run(p["l1"], in1)

    QT = np.concatenate([r1[c]["qt"] for c in range(NC_)], axis=1)  # [C, B*T]
    KT = np.concatenate([r1[c]["kt"] for c in range(NC_)], axis=1)
    V = np.concatenate([r1[c]["v"] for c in range(NC_)], axis=0)  # [B*T, C]

    ones64 = np.ones((1, 64), f32)
    in2 = []
    for c in range(NC_):
        vc = V[:, c * 128 : (c + 1) * 128]  # [B*T, 128] = 2 heads
        # (b, t, p, hl, d) -> sections (b*2+hl) of 16 tiles [128, 65]
        vc5 = vc.reshape(B, 16, 128, 2, 64)
        vaug = np.ones((128, 4 * 16 * 65), f32)
        vw = vaug.reshape(128, 4, 16, 65)
        vw[:, :, :, :64] = vc5.transpose(2, 0, 3, 1, 4).reshape(128, 4, 16, 64)
        in2.append(
            {
                "qt2": np.ascontiguousarray(QT[c * 128 : (c + 1) * 128]),
                "kt2": np.ascontiguousarray(KT[c * 128 : (c + 1) * 128]),
                "vaug": vaug,
                "ones64": ones64,
            }
        )
    r2 = _run(p["l2"], in2)

    attT = np.concatenate([r2[c]["attt"] for c in range(NC_)], axis=0)  # [C, B*T]
    g2 = np.ascontiguousarray(np.broadcast_to(np.asarray(ln2_g, f32), (128, C)))
    b2v = np.ascontiguousarray(np.broadcast_to(np.asarray(ln2_b, f32), (128, C)))
    b1_pre = np.ascontiguousarray(np.asarray(b1, f32).reshape(HID // 128, 128).T)
    bo_ = np.asarray(bo, f32)
    W1a = np.ascontiguousarray(np.asarray(W1, f32).astype(bf16))
    W2a = np.ascontiguousarray(np.asarray(W2, f32).astype(bf16))
    Woa = np.ascontiguousarray(np.asarray(Wo, f32).astype(bf16))

    in3 = []
    for c in range(NC_):
        attc = attT[:, c * ROWS : (c + 1) * ROWS]  # [C, ROWS]
        attc_pre = np.ascontiguousarray(
            attc.reshape(8, 128, ROWS).transpose(1, 0, 2).reshape(128, 8 * ROWS)
        )
        in3.append(
            {
                "attt": attc_pre,
                "xb": np.ascontiguousarray(x_flat[c * ROWS : (c + 1) * ROWS] + bo_),
                "ln_g": g2,
                "ln_b": b2v,
                "wo": Woa,
                "w1": W1a,
                "b1": b1_pre,
                "w2": W2a,
            }
        )
    r3 = _run(p["l3"], in3)

    out = np.concatenate([r3[c]["out"] for c in range(NC_)], axis=0)
    out = out + np.asarray(b2, f32)
    return out.reshape(B, T, C).astype(np.float32)

